# revision 33
# baseline (speedup 1.0000x reference)
"""Trainium2 Bass kernel for BLIF spiking-neuron layer.

Math: the reference's zero-padded-FFT causal conv with kernel
exp(-a_c * t) is exactly the first-order linear recurrence

    v[t] = lam_c * v[t-1] + x[t],   lam_c = exp(-exp(A_log_c))

followed by spike + refractory masking, which reduces to

    s[t]   = (v[t] > 1)
    out[t] = s[t] * (1 - s[t-1])  =  (s[t-1] < s[t])

Sharding: batch B=8 -> one batch per NeuronCore (8 cores), no
cross-core communication. Per core: x_b [T=256, C=128, F=196] f32.

Layout: C on partitions, F(=H*W) on the free dim, T sequential in
graduated chunks (small head/tail chunks shrink pipeline fill/drain).
The host restages each chunk with its even timesteps first and odd
timesteps second, so every on-chip operand is a CONTIGUOUS 2D access
pattern (strided free dims are an order of magnitude slower on DVE).

Work split (the recurrence is unrolled by 2):

    pair-combine (VectorE): y[j]   = lam*x[2j] + x[2j+1]          (1 STT)
    odd chain    (VectorE): v_O[j] = lam^2 * v_O[j-1] + y[j]      (NP STTs)
    even fill    (ScalarE+TensorE):
                   v_E  = copy(x_E) into PSUM        (ACT)
                   v_E += diag(lam) @ v_O_shift       (PE, fp32 exact,
                                                      start=False accum)
    spikes       (ScalarE): s = sign(v - 1) in bf16  (v_E read from PSUM)
    refract mask (VectorE): o = s_prev is_lt s       (bf16, 2x mode)

GpSimd is left idle on purpose: its SBUF ports are shared with VectorE
and any big streaming Pool op halves DVE throughput.

In-DMAs ride the sync HWDGE ring, out-DMAs the scalar ring. Spikes
stream out as bf16 and are converted to f32 on the host.
"""

import sys

for _p in ("/opt/trn_rl_repo", "/root/.axon_site/_ro/trn_rl_repo"):
    if _p not in sys.path:
        sys.path.append(_p)

import numpy as np

import concourse.bacc as bacc
import concourse.bass as bass
import concourse.mybir as mybir
import concourse.tile as tile
from concourse.bass_utils import run_bass_kernel_spmd

T, B, C, H, W = 256, 8, 128, 14, 14
F = H * W          # 196
CHUNKS = [8, 24, 32, 32, 32, 32, 32, 32, 24, 8]
assert sum(CHUNKS) == T and all(c % 2 == 0 for c in CHUNKS)
NCH = len(CHUNKS)
TCMAX = max(CHUNKS)
NPMAX = TCMAX // 2
N_CORES = 8
BANK = 512         # PSUM bank size in f32 elems; fp32 matmul N limit

f32 = mybir.dt.float32
bf16 = mybir.dt.bfloat16
Alu = mybir.AluOpType

_cached_nc = None


def build_program():
    global _cached_nc
    if _cached_nc is not None:
        return _cached_nc

    # Bacc (not raw Bass): its finalize() runs generate_event_semaphores,
    # which splits multi-wait instructions to satisfy the TRN2 limit of one
    # sync wait per instruction (walrus rejects the IR otherwise).
    nc = bacc.Bacc()
    x_ext = nc.declare_dram_parameter("x", [C, T * F], f32, isOutput=False)
    lam_ext = nc.declare_dram_parameter("lam", [C, 2], f32, isOutput=False)
    # [diag(lam) | identity] weight matrices for the even-fill matmuls.
    dmat_ext = nc.declare_dram_parameter("dmat", [C, 2 * C], f32, isOutput=False)
    out_ext = nc.declare_dram_parameter("out", [C, T * F], mybir.dt.uint8, isOutput=True)

    with tile.TileContext(nc) as tc:
        with (
            tc.tile_pool(name="singles", bufs=1) as singles,
            tc.tile_pool(name="xp", bufs=3) as xp,
            tc.tile_pool(name="yp", bufs=2) as yp,
            tc.tile_pool(name="vp", bufs=2) as vp,
            tc.tile_pool(name="sp", bufs=2) as sp,
            tc.tile_pool(name="op", bufs=2) as op,
            tc.tile_pool(name="psum", bufs=2, space=bass.MemorySpace.PSUM) as pp,
        ):
            lam_dma = singles.tile([C, 2], f32)
            nc.sync.dma_start(lam_dma[:], lam_ext[:])
            lam_t = singles.tile([C, 2], f32)
            nc.vector.tensor_copy(out=lam_t[:], in_=lam_dma[:])
            lam1 = lam_t[:, 0:1]
            lam2 = lam_t[:, 1:2]
            neg1 = singles.tile([C, 1], f32)
            nc.vector.memset(neg1[:], -1.0)

            dmat = singles.tile([C, 2 * C], f32)
            nc.sync.dma_start(dmat[:], dmat_ext[:])
            dlam = dmat[:, 0:C]
            ident = dmat[:, C : 2 * C]

            # boundary states for t = -1: v = 0 -> sign(v-1) = -1
            sinit = singles.tile([C, F], bf16)
            nc.vector.memset(sinit[:], -1.0)
            vzero = singles.tile([C, F], f32)
            nc.vector.memset(vzero[:], 0.0)

            prev_v = None   # v[last] of previous chunk (an odd slot)
            prev_s = None   # s[last] of previous chunk
            col = 0         # running column offset into x/out (in elems)
            for k, TCk in enumerate(CHUNKS):
                NP = TCk // 2
                NPF = NP * F

                x_t = xp.tile([C, TCMAX * F], f32)
                nc.sync.dma_start(
                    x_t[:, 0 : TCk * F], x_ext[:, col : col + TCk * F]
                )
                x_E = x_t[:, 0:NPF]
                x_O = x_t[:, NPF : 2 * NPF]

                # y[j] = lam*x[2j] + x[2j+1]
                y_t = yp.tile([C, NPMAX * F], f32)
                nc.vector.scalar_tensor_tensor(
                    out=y_t[:, 0:NPF],
                    in0=x_E,
                    scalar=lam1,
                    in1=x_O,
                    op0=Alu.mult,
                    op1=Alu.add,
                )

                v_O = vp.tile([C, NPMAX * F], f32)

                # odd chain: v[2j+1] = lam^2 * v[2j-1] + y[j]
                src = prev_v
                for j in range(NP):
                    dst = v_O[:, j * F : (j + 1) * F]
                    if src is None:
                        # chunk 0, j=0: v[1] = y[0] (since v[-1] = 0)
                        nc.vector.tensor_copy(out=dst, in_=y_t[:, 0:F])
                    else:
                        nc.vector.scalar_tensor_tensor(
                            out=dst,
                            in0=src,
                            scalar=lam2,
                            in1=y_t[:, j * F : (j + 1) * F],
                            op0=Alu.mult,
                            op1=Alu.add,
                        )
                    src = dst

                # even fill on TensorE (PSUM accumulation, full fp32):
                #   v_E = I @ x_E + diag(lam) @ v_shift,
                #   v_shift = [prev_v | v_O[:-F]]
                # Two double-buffered PSUM halves so PE(k+1) doesn't stall
                # on sign_E(k) draining a single PSUM region.
                s = sp.tile([C, TCMAX * F], bf16)
                s_E = s[:, 0:NPF]
                s_O = s[:, NPF : 2 * NPF]
                carry = vzero[:] if prev_v is None else prev_v
                halfF = (NP // 2) * F
                for h, (glo, ghi) in enumerate(((0, halfF), (halfF, NPF))):
                    hlen = ghi - glo
                    v_E = pp.tile([C, (NPMAX // 2) * F], f32)
                    for llo in range(0, hlen, BANK):
                        lhi = min(llo + BANK, hlen)
                        clo, chi = glo + llo, glo + lhi
                        nc.tensor.matmul(
                            v_E[:, llo:lhi],
                            ident[:],
                            x_E[:, clo:chi],
                            start=True,
                            stop=False,
                        )
                        if clo == 0:
                            nc.tensor.matmul(
                                v_E[:, 0:F], dlam[:], carry, start=False, stop=False
                            )
                            nc.tensor.matmul(
                                v_E[:, F:lhi],
                                dlam[:],
                                v_O[:, 0 : chi - F],
                                start=False,
                                stop=True,
                            )
                        else:
                            nc.tensor.matmul(
                                v_E[:, llo:lhi],
                                dlam[:],
                                v_O[:, clo - F : chi - F],
                                start=False,
                                stop=True,
                            )
                    # spike-encode this half straight out of PSUM
                    nc.scalar.sign(s_E[:, glo:ghi], v_E[:, 0:hlen], bias=neg1[:])

                # s_O = sign(v_O - 1) in {-1, 0, 1}, bf16-exact
                nc.scalar.sign(s_O, v_O[:, 0:NPF], bias=neg1[:])

                # out[t] = (s[t-1] < s[t]), in split layout:
                #   o_O[j] = s_E[j] < s_O[j]
                #   o_E[j] = s_O[j-1] < s_E[j]   (j=0 uses prev chunk's tail)
                o = op.tile([C, TCMAX * F], bf16)
                nc.vector.tensor_tensor(
                    out=o[:, NPF : 2 * NPF], in0=s_E, in1=s_O, op=Alu.is_lt
                )
                if NP > 1:
                    nc.vector.tensor_tensor(
                        out=o[:, F:NPF],
                        in0=s_O[:, 0 : (NP - 1) * F],
                        in1=s_E[:, F:NPF],
                        op=Alu.is_lt,
                    )
                nc.vector.tensor_tensor(
                    out=o[:, 0:F],
                    in0=sinit[:] if prev_s is None else prev_s,
                    in1=s_E[:, 0:F],
                    op=Alu.is_lt,
                )

                # cast spikes to u8 on ScalarE (saturating float->uint) so the
                # store moves 1/2 the bytes of bf16
                o8 = op.tile([C, TCMAX * F], mybir.dt.uint8, tag="o8")
                nc.scalar.copy(o8[:, 0 : TCk * F], o[:, 0 : TCk * F])

                # out-DMAs ride the scalar-engine HWDGE ring so stores don't
                # serialize behind the sync ring's input loads.
                nc.scalar.dma_start(
                    out_ext[:, col : col + TCk * F], o8[:, 0 : TCk * F]
                )

                prev_v = v_O[:, (NP - 1) * F : NPF]
                prev_s = s_O[:, (NP - 1) * F : NPF]
                col += TCk * F

    nc.finalize()
    _cached_nc = nc
    return nc


def _stage_core(xb):
    # xb: [T, C, F] -> [C, T*F] with per-chunk [E-block | O-block] layout
    blocks = []
    t0 = 0
    for TCk in CHUNKS:
        blk = xb[t0 : t0 + TCk]                      # [TCk, C, F]
        blk = blk.reshape(TCk // 2, 2, C, F).transpose(2, 1, 0, 3)
        blocks.append(blk.reshape(C, TCk * F))
        t0 += TCk
    return np.ascontiguousarray(np.concatenate(blocks, axis=1))


def _unstage_core(ob):
    # ob: [C, T*F] f32 -> [T, C, F]
    parts = []
    c0 = 0
    for TCk in CHUNKS:
        blk = ob[:, c0 : c0 + TCk * F].reshape(C, 2, TCk // 2, F)
        parts.append(blk.transpose(2, 1, 0, 3).reshape(TCk, C, F))
        c0 += TCk * F
    return np.concatenate(parts, axis=0)


def make_in_maps(x, A_log):
    lam = np.exp(-np.exp(A_log.astype(np.float64))).reshape(C)
    lam_pair = np.stack([lam, lam * lam], axis=1).astype(np.float32)  # [C, 2]
    dmat = np.concatenate(
        [np.diag(lam.astype(np.float32)), np.eye(C, dtype=np.float32)], axis=1
    )
    maps = []
    for b in range(B):
        xb = _stage_core(x[:, b].reshape(T, C, F))
        maps.append({"x": xb, "lam": lam_pair, "dmat": dmat})
    return maps


def gather_output(results):
    outs = []
    for b in range(B):
        o = np.asarray(results[b]["out"]).astype(np.float32)  # [C, T*F] u8->f32
        outs.append(_unstage_core(o))
    return np.stack(outs, axis=1).reshape(T, B, C, H, W)


def kernel(x, A_log):
    nc = build_program()
    in_maps = make_in_maps(x, A_log)
    res = run_bass_kernel_spmd(nc, in_maps, list(range(N_CORES)))
    return gather_output(res.results)
